# revision 2
# baseline (speedup 1.0000x reference)
"""Self-attention layer (softmax(X @ X^T) @ X) on 8 Trainium2 NeuronCores.

Data-parallel over batch: each of the 8 cores computes one batch element's
full attention for X of shape [2048, 512].

Per-core algorithm. Scores are computed TRANSPOSED (T[j, i] = <x_i, x_j>,
key index j on partitions) so the exponentiated tile is already in the
layout the PV matmul needs as its stationary operand. QK matmuls run in
fp8-e4m3 DoubleRow (2 MACs/cell/cycle, K=256 per matmul) — score rounding
cancels exactly through the l-normalization. PV runs in bf16 for output
precision. The softmax stabilizer c[i] = |x_i|^2 is subtracted on the
vector engine from a pre-broadcast [128, S] row; l and 1/l come from an
N=1 matmul sharing the PV matmuls' loaded weights.

  1. Input streams in via 4 batched DMAs (4 row-tiles each) split across
     the Sync and Activation HWDGE queues, issued before any other work so
     the transfer pipeline fills during the framework preamble.
  2. Per tile (as its batch lands): X_bf = bf16(X) on gpsimd (keeps the
     scalar engine free for exp); PE-transpose X_bf into Xt8 (fp8, [d, s]).
  3. negc for chunk n is computed as soon as tiles 4n..4n+3 are in
     (sq = Xt8*Xt8; c row via ones-matmul over sq; broadcast -c into
     negc_full [128, S] via K=1 matmul + copy) so no QK tile ever waits on
     it mid-kernel.
  4. For each query chunk ic (512 queries), for each key block jt:
       psum = QK fp8 DoubleRow matmuls (2 x K=256)
       psum -= c (vector engine)
       E_T[jt block][:, ic] = exp(psum)   (scalar engine, direct to SBUF)
     qk_psum has 3 banks so the add+exp drain of a tile is fully hidden
     behind two tile periods of PE work.
  5. PV per query block i (software-pipelined one chunk behind QK):
       po = sum_j E_T[:, j-block, i]^T @ X_bf[j]   (bf16)
       l  = same loaded weights @ ones (N=1);  O_i = po / l; DMA out.
"""

import os
import numpy as np

B, S, D = 8, 2048, 512
P = 128
NI = S // P  # 16 row blocks
NK = D // P  # 4 d-tiles
JC = 512     # query column chunk (one psum bank)
NC = S // JC  # 4 chunks
NSUB = JC // P  # 4 i-tiles per chunk
LB = 4       # input tiles per batched DMA
NB = NI // LB  # 4 batched input DMAs

_CACHE = {}


def _build_nc():
    from contextlib import ExitStack

    import concourse.bacc as bacc
    import concourse.mybir as mybir
    import concourse.tile as tile
    from concourse import masks

    f32 = mybir.dt.float32
    bf16 = mybir.dt.bfloat16
    fp8 = mybir.dt.float8e4
    AF = mybir.ActivationFunctionType
    DR = mybir.MatmulPerfMode.DoubleRow

    nc = bacc.Bacc("TRN2", target_bir_lowering=False, debug=False, num_devices=B)
    inp = nc.dram_tensor("inputs", [S, D], f32, kind="ExternalInput").ap()
    out = nc.dram_tensor("out", [S, D], f32, kind="ExternalOutput").ap()

    with tile.TileContext(nc) as tc, ExitStack() as ctx:
        const_pool = ctx.enter_context(tc.tile_pool(name="const", bufs=1))
        persist = ctx.enter_context(tc.tile_pool(name="persist", bufs=1))
        stat_pool = ctx.enter_context(tc.tile_pool(name="stat", bufs=2))
        osb_pool = ctx.enter_context(tc.tile_pool(name="osb", bufs=2))
        # PSUM budget (8 banks): qk 3 + pt 2 + pv 2 + lc 1. Three QK banks
        # hide a tile's add+exp drain (~1.7us) behind two interleaved tile
        # periods (~2.8us); PV's drain (recip+mul, ~1.4us) fits easily in
        # two i-block periods (~7.7us), so 2 banks suffice there.
        qk_psum = ctx.enter_context(tc.tile_pool(name="qk_psum", bufs=3, space="PSUM"))
        tr_psum = ctx.enter_context(tc.tile_pool(name="tr_psum", bufs=2, space="PSUM"))
        pv_psum = ctx.enter_context(tc.tile_pool(name="pv_psum", bufs=2, space="PSUM"))
        l_psum = ctx.enter_context(tc.tile_pool(name="l_psum", bufs=1, space="PSUM"))

        # X_f32 is the DMA landing buffer for the whole input; batched DMAs
        # are issued before anything else so transfers overlap the preamble.
        X_f32 = persist.tile([P, NI * D], f32, tag="xf32", name="xf32")
        Xf3 = X_f32[:].rearrange("p (t d) -> p t d", t=NI)
        inp3 = inp.rearrange("(t p) d -> p t d", t=NI)
        for b in range(NB):
            eng = nc.sync if b % 2 == 0 else nc.scalar
            eng.dma_start(
                Xf3[:, b * LB : (b + 1) * LB], inp3[:, b * LB : (b + 1) * LB]
            )

        ident = const_pool.tile([P, P], bf16, tag="ident", name="ident")
        masks.make_identity(nc, ident[:])
        ones_row = const_pool.tile([1, P], bf16, tag="ones_row", name="ones_row")
        nc.vector.memset(ones_row[:], 1.0)
        ones_col = const_pool.tile([P, 1], bf16, tag="ones_col", name="ones_col")
        nc.vector.memset(ones_col[:], 1.0)

        X_bf = persist.tile([P, NI * D], bf16, tag="xbf", name="xbf")
        Xt8 = persist.tile([P, NK * S], fp8, tag="xt8", name="xt8")
        sq = persist.tile([P, NK * S], bf16, tag="sq", name="sq")
        negc = persist.tile([1, S], bf16, tag="negc", name="negc")
        negc_full = persist.tile([P, S], bf16, tag="negc_full", name="negc_full")
        E_T = persist.tile([P, NI * S], bf16, tag="et", name="et")

        Xt8_3 = Xt8[:].rearrange("p (k s) -> p k s", k=NK)
        Xt8_4 = Xt8[:].rearrange("p (k2 two s) -> p k2 two s", k2=NK // 2, two=2)
        sq3 = sq[:].rearrange("p (k s) -> p k s", k=NK)

        # ---- emit helpers ----
        def emit_load_tile(i):
            dcols = slice(i * D, (i + 1) * D)
            nc.gpsimd.tensor_copy(X_bf[:, dcols], Xf3[:, i])
            pt = tr_psum.tile([P, NK, P], bf16, tag="pt", name=f"ptx{i}")
            for k in range(NK):
                nc.tensor.matmul(
                    pt[:, k],
                    lhsT=X_bf[:, i * D + k * P : i * D + (k + 1) * P],
                    rhs=ident[:],
                    is_transpose=True,
                    skip_group_check=True,
                )
            nc.vector.tensor_copy(Xt8_3[:, :, i * P : (i + 1) * P], pt[:])

        def emit_sq_negc(ic):
            # c[s] = sum_d X[s, d]^2 for chunk ic's columns (psum row),
            # negated and broadcast to all partitions of negc_full
            ccols = slice(ic * JC, (ic + 1) * JC)
            for k in range(NK):
                eng = nc.gpsimd if k % 2 == 0 else nc.vector
                eng.tensor_mul(
                    sq3[:, k, ccols], Xt8_3[:, k, ccols], Xt8_3[:, k, ccols]
                )
            pc = tr_psum.tile([1, JC], f32, tag="pt", name=f"c{ic}")
            for k in range(NK):
                nc.tensor.matmul(
                    pc[:],
                    lhsT=ones_col[:],
                    rhs=sq3[:, k, ccols],
                    start=(k == 0),
                    stop=(k == NK - 1),
                )
            nc.vector.tensor_scalar_mul(negc[:, ccols], pc[:], -1.0)
            pb = tr_psum.tile([P, JC], f32, tag="pt", name=f"pb{ic}")
            nc.tensor.matmul(pb[:], lhsT=ones_row[:], rhs=negc[:, ccols])
            nc.vector.tensor_copy(negc_full[:, ccols], pb[:])

        def emit_qk_tile(ic, jt):
            ccols = slice(ic * JC, (ic + 1) * JC)
            ps = qk_psum.tile([P, JC], f32, tag="qk", name=f"qk{ic}_{jt}")
            for k2 in range(NK // 2):
                nc.tensor.matmul(
                    ps[:],
                    lhsT=Xt8_4[:, k2, :, jt * P : (jt + 1) * P],
                    rhs=Xt8_4[:, k2, :, ccols],
                    perf_mode=DR,
                    start=(k2 == 0),
                    stop=(k2 == NK // 2 - 1),
                )
            nc.vector.tensor_add(ps[:], ps[:], negc_full[:, ccols])
            nc.scalar.activation(
                E_T[:, jt * S + ic * JC : jt * S + (ic + 1) * JC],
                ps[:],
                AF.Exp,
            )

        # ---- startup: process tiles as their batch DMAs land. negc for
        # chunk n is emitted right after tile 4n+3 so it's always ready
        # before any consumer; chunk 0's QK tiles interleave one tile
        # behind the loads so each QK tile's transposed operand has already
        # drained when the tensor engine reaches it. ----
        for i in range(NI):
            emit_load_tile(i)
            if i % LB == LB - 1:
                emit_sq_negc(i // LB)
            if i == NSUB:
                for jt in range(NSUB):
                    emit_qk_tile(0, jt)
            elif i > NSUB:
                emit_qk_tile(0, i - 1)
        emit_qk_tile(0, NI - 1)

        def emit_pv_steps(i, po, pl, j0, j1):
            for j in range(j0, j1):
                lhsT = E_T[:, j * S + i * P : j * S + (i + 1) * P]
                nc.tensor.matmul(
                    po[:],
                    lhsT=lhsT,
                    rhs=X_bf[:, j * D : (j + 1) * D],
                    start=(j == 0),
                    stop=(j == NI - 1),
                )
                nc.tensor.matmul(
                    pl[:],
                    lhsT=lhsT,
                    rhs=ones_col[:],
                    start=(j == 0),
                    stop=(j == NI - 1),
                )

        def emit_pv_end(i, po, pl):
            linv = stat_pool.tile([P, 1], f32, tag="linv", name=f"linv{i}")
            nc.vector.reciprocal(linv[:], pl[:])
            osb = osb_pool.tile([P, D], f32, tag="osb", name=f"osb{i}")
            nc.vector.tensor_scalar_mul(osb[:], po[:], linv[:])
            nc.sync.dma_start(out[i * P : (i + 1) * P, :], osb[:])

        # Main loop: chunk ic's QK tiles are interleaved with chunk ic-1's PV
        # matmuls at quarter-tile granularity, so the tensor engine always has
        # ready PV work queued while a QK psum bank waits on its add+exp
        # drain. The PV j-step order matches the exp completion order of the
        # previous chunk, so interleaved steps never wait on the softmax.
        # (Chunk 0's QK was emitted during the load stream above.)
        po = pl = None
        for ic in range(1, NC + 1):
            for jt in range(NI):
                if ic < NC:
                    emit_qk_tile(ic, jt)
                i = (ic - 1) * NSUB + jt // NSUB
                if jt % NSUB == 0:
                    po = pv_psum.tile([P, D], f32, tag="pv", name=f"pv{i}")
                    pl = l_psum.tile([P, 1], f32, tag="lc", name=f"l{i}")
                emit_pv_steps(i, po, pl, (jt % NSUB) * NSUB, (jt % NSUB + 1) * NSUB)
                if jt % NSUB == NSUB - 1:
                    emit_pv_end(i, po, pl)

    nc.compile()
    return nc


def _maybe_install_trace_hook():
    """Install the NTFF profile hook (test/profiling only; optional)."""
    import sys
    import types

    try:
        from antenv.axon_hooks import get_axon_ntff_profile_hook  # noqa: F401

        return  # already available
    except ImportError:
        pass
    try:
        mod = types.ModuleType("antenv.axon_hooks")
        _hook = [None]
        mod.set_axon_ntff_profile_hook = lambda h: _hook.__setitem__(0, h)
        mod.get_axon_ntff_profile_hook = lambda: _hook[0]
        sys.modules["antenv.axon_hooks"] = mod
        import antenv

        antenv.axon_hooks = mod
        from trn_agent_boot.trn_boot import _ntff_profile_via_ctypes

        mod.set_axon_ntff_profile_hook(
            _ntff_profile_via_ctypes("/opt/axon/libaxon_pjrt.so")
        )
    except Exception:
        pass


def kernel(inputs: np.ndarray) -> np.ndarray:
    from concourse.bass_utils import run_bass_kernel_spmd

    x = np.ascontiguousarray(np.asarray(inputs, dtype=np.float32))
    assert x.shape == (B, S, D), f"unexpected input shape {x.shape}"

    if "nc" not in _CACHE:
        _CACHE["nc"] = _build_nc()
    nc = _CACHE["nc"]

    trace = bool(int(os.environ.get("ATT_KERNEL_TRACE", "0")))
    if trace:
        _maybe_install_trace_hook()

    in_maps = [{"inputs": x[b]} for b in range(B)]
    res = run_bass_kernel_spmd(nc, in_maps, core_ids=list(range(B)), trace=trace)
    kernel.last_exec_time_ns = res.exec_time_ns
    return np.stack([res.results[b]["out"] for b in range(B)], axis=0)


kernel.last_exec_time_ns = None


# revision 3
# speedup vs baseline: 1.1232x; 1.1232x over previous
"""Self-attention layer (softmax(X @ X^T) @ X) on 8 Trainium2 NeuronCores.

Data-parallel over batch: each of the 8 cores computes one batch element's
full attention for X of shape [2048, 512].

Per-core algorithm. Scores are computed TRANSPOSED (T[j, i] = <x_i, x_j>,
key index j on partitions) so the exponentiated tile is already in the
layout the PV matmul needs as its stationary operand. QK matmuls run in
fp8-e4m3 DoubleRow (2 MACs/cell/cycle, K=256 per matmul) — score rounding
cancels exactly through the l-normalization. PV runs in bf16 for output
precision. The softmax stabilizer c[i] = |x_i|^2 is subtracted on the
vector engine from a pre-broadcast [128, S] row; l and 1/l come from an
N=1 matmul sharing the PV matmuls' loaded weights.

  1. Input streams in via 4 batched DMAs (4 row-tiles each) split across
     the Sync and Activation HWDGE queues, issued before any other work so
     the transfer pipeline fills during the framework preamble.
  2. Per tile (as its batch lands): X_bf = bf16(X) on gpsimd (keeps the
     scalar engine free for exp); PE-transpose X_bf into Xt8 (fp8, [d, s]).
  3. negc for chunk n is computed as soon as tiles 4n..4n+3 are in
     (sq = Xt8*Xt8; c row via ones-matmul over sq; broadcast -c into
     negc_full [128, S] via K=1 matmul + copy) so no QK tile ever waits on
     it mid-kernel.
  4. For each query chunk ic (512 queries), for each key block jt:
       psum = QK fp8 DoubleRow matmuls (2 x K=256)
       psum -= c (vector engine)
       E_T[jt block][:, ic] = exp(psum)   (scalar engine, direct to SBUF)
     qk_psum has 3 banks so the add+exp drain of a tile is fully hidden
     behind two tile periods of PE work.
  5. PV per query block i (software-pipelined one chunk behind QK):
       po = sum_j E_T[:, j-block, i]^T @ X_bf[j]   (bf16)
       l  = same loaded weights @ ones (N=1);  O_i = po / l; DMA out.
"""

import os
import numpy as np

B, S, D = 8, 2048, 512
P = 128
NI = S // P  # 16 row blocks
NK = D // P  # 4 d-tiles
JC = 512     # query column chunk (one psum bank)
NC = S // JC  # 4 chunks
NSUB = JC // P  # 4 i-tiles per chunk
LB = 4       # input tiles per batched DMA
NB = NI // LB  # 4 batched input DMAs

_CACHE = {}


def _build_nc():
    from contextlib import ExitStack

    import concourse.bacc as bacc
    import concourse.mybir as mybir
    import concourse.tile as tile
    from concourse import masks

    f32 = mybir.dt.float32
    bf16 = mybir.dt.bfloat16
    fp8 = mybir.dt.float8e4
    AF = mybir.ActivationFunctionType
    DR = mybir.MatmulPerfMode.DoubleRow

    nc = bacc.Bacc("TRN2", target_bir_lowering=False, debug=False, num_devices=B)
    inp = nc.dram_tensor("inputs", [S, D], f32, kind="ExternalInput").ap()
    out = nc.dram_tensor("out", [S, D], f32, kind="ExternalOutput").ap()

    with tile.TileContext(nc) as tc, ExitStack() as ctx:
        const_pool = ctx.enter_context(tc.tile_pool(name="const", bufs=1))
        persist = ctx.enter_context(tc.tile_pool(name="persist", bufs=1))
        stat_pool = ctx.enter_context(tc.tile_pool(name="stat", bufs=2))
        osb_pool = ctx.enter_context(tc.tile_pool(name="osb", bufs=2))
        # PSUM budget (8 banks): qk 3 + pt 2 + pv 2 + lc 1. Three QK banks
        # hide a tile's add+exp drain (~1.7us) behind two interleaved tile
        # periods (~2.8us); PV's drain (recip+mul, ~1.4us) fits easily in
        # two i-block periods (~7.7us), so 2 banks suffice there.
        qk_psum = ctx.enter_context(tc.tile_pool(name="qk_psum", bufs=3, space="PSUM"))
        tr_psum = ctx.enter_context(tc.tile_pool(name="tr_psum", bufs=2, space="PSUM"))
        pv_psum = ctx.enter_context(tc.tile_pool(name="pv_psum", bufs=2, space="PSUM"))
        l_psum = ctx.enter_context(tc.tile_pool(name="l_psum", bufs=1, space="PSUM"))

        # X_f32 is the DMA landing buffer for the whole input; batched DMAs
        # are issued before anything else so transfers overlap the preamble.
        X_f32 = persist.tile([P, NI * D], f32, tag="xf32", name="xf32")
        Xf3 = X_f32[:].rearrange("p (t d) -> p t d", t=NI)
        inp3 = inp.rearrange("(t p) d -> p t d", t=NI)
        for b in range(NB):
            eng = nc.sync if b % 2 == 0 else nc.scalar
            eng.dma_start(
                Xf3[:, b * LB : (b + 1) * LB], inp3[:, b * LB : (b + 1) * LB]
            )

        ident = const_pool.tile([P, P], bf16, tag="ident", name="ident")
        masks.make_identity(nc, ident[:])
        ones_row = const_pool.tile([1, P], bf16, tag="ones_row", name="ones_row")
        nc.vector.memset(ones_row[:], 1.0)
        ones_col = const_pool.tile([P, 1], bf16, tag="ones_col", name="ones_col")
        nc.vector.memset(ones_col[:], 1.0)

        X_bf = persist.tile([P, NI * D], bf16, tag="xbf", name="xbf")
        Xt8 = persist.tile([P, NK * S], fp8, tag="xt8", name="xt8")
        sq = persist.tile([P, NK * S], bf16, tag="sq", name="sq")
        negc = persist.tile([1, S], bf16, tag="negc", name="negc")
        negc_full = persist.tile([P, S], bf16, tag="negc_full", name="negc_full")
        E_T = persist.tile([P, NI * S], bf16, tag="et", name="et")

        Xt8_3 = Xt8[:].rearrange("p (k s) -> p k s", k=NK)
        Xt8_4 = Xt8[:].rearrange("p (k2 two s) -> p k2 two s", k2=NK // 2, two=2)
        sq3 = sq[:].rearrange("p (k s) -> p k s", k=NK)

        # ---- emit helpers ----
        def emit_load_tile(i):
            dcols = slice(i * D, (i + 1) * D)
            nc.scalar.copy(X_bf[:, dcols], Xf3[:, i])
            pt = tr_psum.tile([P, NK, P], bf16, tag="pt", name=f"ptx{i}")
            for k in range(NK):
                nc.tensor.matmul(
                    pt[:, k],
                    lhsT=X_bf[:, i * D + k * P : i * D + (k + 1) * P],
                    rhs=ident[:],
                    is_transpose=True,
                    skip_group_check=True,
                )
            nc.vector.tensor_copy(Xt8_3[:, :, i * P : (i + 1) * P], pt[:])

        def emit_sq_negc(ic):
            # c[s] = sum_d X[s, d]^2 for chunk ic's columns (psum row),
            # negated and broadcast to all partitions of negc_full
            ccols = slice(ic * JC, (ic + 1) * JC)
            for k in range(NK):
                eng = nc.gpsimd if k % 2 == 0 else nc.vector
                eng.tensor_mul(
                    sq3[:, k, ccols], Xt8_3[:, k, ccols], Xt8_3[:, k, ccols]
                )
            pc = tr_psum.tile([1, JC], f32, tag="pt", name=f"c{ic}")
            for k in range(NK):
                nc.tensor.matmul(
                    pc[:],
                    lhsT=ones_col[:],
                    rhs=sq3[:, k, ccols],
                    start=(k == 0),
                    stop=(k == NK - 1),
                )
            nc.vector.tensor_scalar_mul(negc[:, ccols], pc[:], -1.0)
            pb = tr_psum.tile([P, JC], f32, tag="pt", name=f"pb{ic}")
            nc.tensor.matmul(pb[:], lhsT=ones_row[:], rhs=negc[:, ccols])
            nc.vector.tensor_copy(negc_full[:, ccols], pb[:])

        def emit_qk_tile(ic, jt):
            ccols = slice(ic * JC, (ic + 1) * JC)
            ps = qk_psum.tile([P, JC], f32, tag="qk", name=f"qk{ic}_{jt}")
            for k2 in range(NK // 2):
                nc.tensor.matmul(
                    ps[:],
                    lhsT=Xt8_4[:, k2, :, jt * P : (jt + 1) * P],
                    rhs=Xt8_4[:, k2, :, ccols],
                    perf_mode=DR,
                    start=(k2 == 0),
                    stop=(k2 == NK // 2 - 1),
                )
            nc.vector.tensor_add(ps[:], ps[:], negc_full[:, ccols])
            nc.scalar.activation(
                E_T[:, jt * S + ic * JC : jt * S + (ic + 1) * JC],
                ps[:],
                AF.Exp,
            )

        # ---- startup: process tiles as their batch DMAs land. negc for
        # chunk n is emitted right after tile 4n+3 so it's always ready
        # before any consumer; chunk 0's QK tiles interleave one tile
        # behind the loads so each QK tile's transposed operand has already
        # drained when the tensor engine reaches it. ----
        for i in range(NI):
            emit_load_tile(i)
            if i % LB == LB - 1:
                emit_sq_negc(i // LB)
            if i == NSUB:
                for jt in range(NSUB):
                    emit_qk_tile(0, jt)
            elif i > NSUB:
                emit_qk_tile(0, i - 1)
        emit_qk_tile(0, NI - 1)

        def emit_pv_steps(i, po, pl, j0, j1):
            for j in range(j0, j1):
                lhsT = E_T[:, j * S + i * P : j * S + (i + 1) * P]
                nc.tensor.matmul(
                    po[:],
                    lhsT=lhsT,
                    rhs=X_bf[:, j * D : (j + 1) * D],
                    start=(j == 0),
                    stop=(j == NI - 1),
                )
                nc.tensor.matmul(
                    pl[:],
                    lhsT=lhsT,
                    rhs=ones_col[:],
                    start=(j == 0),
                    stop=(j == NI - 1),
                )

        def emit_pv_end(i, po, pl):
            linv = stat_pool.tile([P, 1], f32, tag="linv", name=f"linv{i}")
            nc.vector.reciprocal(linv[:], pl[:])
            osb = osb_pool.tile([P, D], f32, tag="osb", name=f"osb{i}")
            nc.vector.tensor_scalar_mul(osb[:], po[:], linv[:])
            nc.sync.dma_start(out[i * P : (i + 1) * P, :], osb[:])

        # Main loop: chunk ic's QK tiles are interleaved with chunk ic-1's PV
        # matmuls at quarter-tile granularity, so the tensor engine always has
        # ready PV work queued while a QK psum bank waits on its add+exp
        # drain. The PV j-step order matches the exp completion order of the
        # previous chunk, so interleaved steps never wait on the softmax.
        # (Chunk 0's QK was emitted during the load stream above.)
        po = pl = None
        for ic in range(1, NC + 1):
            for jt in range(NI):
                if ic < NC:
                    emit_qk_tile(ic, jt)
                i = (ic - 1) * NSUB + jt // NSUB
                if jt % NSUB == 0:
                    po = pv_psum.tile([P, D], f32, tag="pv", name=f"pv{i}")
                    pl = l_psum.tile([P, 1], f32, tag="lc", name=f"l{i}")
                emit_pv_steps(i, po, pl, (jt % NSUB) * NSUB, (jt % NSUB + 1) * NSUB)
                if jt % NSUB == NSUB - 1:
                    emit_pv_end(i, po, pl)

    nc.compile()
    return nc


def _maybe_install_trace_hook():
    """Install the NTFF profile hook (test/profiling only; optional)."""
    import sys
    import types

    try:
        from antenv.axon_hooks import get_axon_ntff_profile_hook  # noqa: F401

        return  # already available
    except ImportError:
        pass
    try:
        mod = types.ModuleType("antenv.axon_hooks")
        _hook = [None]
        mod.set_axon_ntff_profile_hook = lambda h: _hook.__setitem__(0, h)
        mod.get_axon_ntff_profile_hook = lambda: _hook[0]
        sys.modules["antenv.axon_hooks"] = mod
        import antenv

        antenv.axon_hooks = mod
        from trn_agent_boot.trn_boot import _ntff_profile_via_ctypes

        mod.set_axon_ntff_profile_hook(
            _ntff_profile_via_ctypes("/opt/axon/libaxon_pjrt.so")
        )
    except Exception:
        pass


def kernel(inputs: np.ndarray) -> np.ndarray:
    from concourse.bass_utils import run_bass_kernel_spmd

    x = np.ascontiguousarray(np.asarray(inputs, dtype=np.float32))
    assert x.shape == (B, S, D), f"unexpected input shape {x.shape}"

    if "nc" not in _CACHE:
        _CACHE["nc"] = _build_nc()
    nc = _CACHE["nc"]

    trace = bool(int(os.environ.get("ATT_KERNEL_TRACE", "0")))
    if trace:
        _maybe_install_trace_hook()

    in_maps = [{"inputs": x[b]} for b in range(B)]
    res = run_bass_kernel_spmd(nc, in_maps, core_ids=list(range(B)), trace=trace)
    kernel.last_exec_time_ns = res.exec_time_ns
    return np.stack([res.results[b]["out"] for b in range(B)], axis=0)


kernel.last_exec_time_ns = None


# revision 7
# speedup vs baseline: 1.1365x; 1.0118x over previous
"""Self-attention layer (softmax(X @ X^T) @ X) on 8 Trainium2 NeuronCores.

Data-parallel over batch: each of the 8 cores computes one batch element's
full attention for X of shape [2048, 512].

Per-core algorithm. Scores are computed TRANSPOSED (T[j, i] = <x_i, x_j>,
key index j on partitions) so the exponentiated tile is already in the
layout the PV matmul needs as its stationary operand. QK matmuls run in
fp8-e4m3 DoubleRow (2 MACs/cell/cycle, K=256 per matmul) — score rounding
cancels exactly through the l-normalization. PV runs in bf16 for output
precision. The softmax stabilizer c[i] = |x_i|^2 is subtracted on the
vector engine from a pre-broadcast [128, S] row; l and 1/l come from an
N=1 matmul sharing the PV matmuls' loaded weights.

  1. Input streams in via 4 batched DMAs (4 row-tiles each) split across
     the Sync and Activation HWDGE queues, issued before any other work so
     the transfer pipeline fills during the framework preamble.
  2. Per tile (as its batch lands): X_bf = bf16(X) on gpsimd (keeps the
     scalar engine free for exp); PE-transpose X_bf into Xt8 (fp8, [d, s]).
  3. negc for chunk n is computed as soon as tiles 4n..4n+3 are in
     (sq = Xt8*Xt8; c row via ones-matmul over sq; broadcast -c into
     negc_full [128, S] via K=1 matmul + copy) so no QK tile ever waits on
     it mid-kernel.
  4. For each query chunk ic (512 queries), for each key block jt:
       psum = QK fp8 DoubleRow matmuls (2 x K=256)
       psum -= c (vector engine)
       E_T[jt block][:, ic] = exp(psum)   (scalar engine, direct to SBUF)
     qk_psum has 3 banks so the add+exp drain of a tile is fully hidden
     behind two tile periods of PE work.
  5. PV per query block i (software-pipelined one chunk behind QK):
       po = sum_j E_T[:, j-block, i]^T @ X_bf[j]   (bf16)
       l  = same loaded weights @ ones (N=1);  O_i = po / l; DMA out.
"""

import os
import numpy as np

B, S, D = 8, 2048, 512
P = 128
NI = S // P  # 16 row blocks
NK = D // P  # 4 d-tiles
JC = 512     # query column chunk (one psum bank)
NC = S // JC  # 4 chunks
NSUB = JC // P  # 4 i-tiles per chunk
LB = 2       # input tiles per batched DMA
NB = NI // LB  # 8 batched input DMAs

_CACHE = {}


def _build_nc():
    from contextlib import ExitStack

    import concourse.bacc as bacc
    import concourse.mybir as mybir
    import concourse.tile as tile
    from concourse import masks

    f32 = mybir.dt.float32
    bf16 = mybir.dt.bfloat16
    fp8 = mybir.dt.float8e4
    AF = mybir.ActivationFunctionType
    DR = mybir.MatmulPerfMode.DoubleRow

    nc = bacc.Bacc("TRN2", target_bir_lowering=False, debug=False, num_devices=B)
    inp = nc.dram_tensor("inputs", [S, D], f32, kind="ExternalInput").ap()
    out = nc.dram_tensor("out", [S, D], f32, kind="ExternalOutput").ap()

    with tile.TileContext(nc) as tc, ExitStack() as ctx:
        const_pool = ctx.enter_context(tc.tile_pool(name="const", bufs=1))
        persist = ctx.enter_context(tc.tile_pool(name="persist", bufs=1))
        stat_pool = ctx.enter_context(tc.tile_pool(name="stat", bufs=2))
        osb_pool = ctx.enter_context(tc.tile_pool(name="osb", bufs=2))
        # PSUM budget (8 banks): qk 3 + pt 1 + pv 3 + lc 1. Three QK banks
        # hide a tile's add+exp drain (~1.7us) behind two interleaved tile
        # periods (~2.8us); three PV banks keep po issue stall-free across
        # i-block boundaries. tr only carries the load-phase transposes and
        # negc scratch, which the DMA stream paces anyway.
        qk_psum = ctx.enter_context(tc.tile_pool(name="qk_psum", bufs=3, space="PSUM"))
        tr_psum = ctx.enter_context(tc.tile_pool(name="tr_psum", bufs=1, space="PSUM"))
        pv_psum = ctx.enter_context(tc.tile_pool(name="pv_psum", bufs=3, space="PSUM"))
        l_psum = ctx.enter_context(tc.tile_pool(name="l_psum", bufs=1, space="PSUM"))

        # X_f32 is the DMA landing buffer for the whole input; batched DMAs
        # are issued before anything else so transfers overlap the preamble.
        X_f32 = persist.tile([P, NI * D], f32, tag="xf32", name="xf32")
        Xf3 = X_f32[:].rearrange("p (t d) -> p t d", t=NI)
        inp3 = inp.rearrange("(t p) d -> p t d", t=NI)
        for b in range(NB):
            eng = nc.sync if b % 2 == 0 else nc.scalar
            eng.dma_start(
                Xf3[:, b * LB : (b + 1) * LB], inp3[:, b * LB : (b + 1) * LB]
            )

        # PE warm-up: HAM keeps the PE clock-gated at 1.2 GHz until it sees
        # ~3.4us of sustained matmul activity, and transpose-mode matmuls
        # don't count. Burn a burst of dummy N=128 matmuls on a memset tile
        # while the input DMA is still in flight so every load-phase matmul
        # (transposes, chunk-0 QK) runs at the full 2.4 GHz.
        warm = const_pool.tile([P, P], bf16, tag="warm", name="warm")
        nc.vector.memset(warm[:], 0.0)
        wps = l_psum.tile([P, P], f32, tag="lc", name="warmps")
        for w in range(40):
            nc.tensor.matmul(wps[:], lhsT=warm[:], rhs=warm[:], skip_group_check=True)

        ident = const_pool.tile([P, P], bf16, tag="ident", name="ident")
        masks.make_identity(nc, ident[:])
        ones_row = const_pool.tile([1, P], bf16, tag="ones_row", name="ones_row")
        nc.vector.memset(ones_row[:], 1.0)
        ones_col = const_pool.tile([P, 1], bf16, tag="ones_col", name="ones_col")
        nc.vector.memset(ones_col[:], 1.0)

        X_bf = persist.tile([P, NI * D], bf16, tag="xbf", name="xbf")
        Xt8 = persist.tile([P, NK * S], fp8, tag="xt8", name="xt8")
        sq = persist.tile([P, NK * S], bf16, tag="sq", name="sq")
        negc = persist.tile([1, S], bf16, tag="negc", name="negc")
        negc_full = persist.tile([P, S], bf16, tag="negc_full", name="negc_full")
        E_T = persist.tile([P, NI * S], bf16, tag="et", name="et")

        Xt8_3 = Xt8[:].rearrange("p (k s) -> p k s", k=NK)
        Xt8_4 = Xt8[:].rearrange("p (k2 two s) -> p k2 two s", k2=NK // 2, two=2)
        sq3 = sq[:].rearrange("p (k s) -> p k s", k=NK)

        # ---- emit helpers ----
        def emit_load_tile(i):
            dcols = slice(i * D, (i + 1) * D)
            nc.scalar.copy(X_bf[:, dcols], Xf3[:, i])
            pt = tr_psum.tile([P, NK, P], bf16, tag="pt", name=f"ptx{i}")
            for k in range(NK):
                nc.tensor.matmul(
                    pt[:, k],
                    lhsT=X_bf[:, i * D + k * P : i * D + (k + 1) * P],
                    rhs=ident[:],
                    is_transpose=True,
                    skip_group_check=True,
                )
            nc.vector.tensor_copy(Xt8_3[:, :, i * P : (i + 1) * P], pt[:])

        def emit_sq_negc(ic):
            # c[s] = sum_d X[s, d]^2 for chunk ic's columns (psum row),
            # negated and broadcast to all partitions of negc_full
            ccols = slice(ic * JC, (ic + 1) * JC)
            for k in range(NK):
                eng = nc.gpsimd if k % 2 == 0 else nc.vector
                eng.tensor_mul(
                    sq3[:, k, ccols], Xt8_3[:, k, ccols], Xt8_3[:, k, ccols]
                )
            pc = tr_psum.tile([1, JC], f32, tag="pt", name=f"c{ic}")
            for k in range(NK):
                nc.tensor.matmul(
                    pc[:],
                    lhsT=ones_col[:],
                    rhs=sq3[:, k, ccols],
                    start=(k == 0),
                    stop=(k == NK - 1),
                )
            nc.vector.tensor_scalar_mul(negc[:, ccols], pc[:], -1.0)
            pb = tr_psum.tile([P, JC], f32, tag="pt", name=f"pb{ic}")
            nc.tensor.matmul(pb[:], lhsT=ones_row[:], rhs=negc[:, ccols])
            nc.vector.tensor_copy(negc_full[:, ccols], pb[:])

        def emit_qk_tile(ic, jt):
            ccols = slice(ic * JC, (ic + 1) * JC)
            ps = qk_psum.tile([P, JC], f32, tag="qk", name=f"qk{ic}_{jt}")
            for k2 in range(NK // 2):
                nc.tensor.matmul(
                    ps[:],
                    lhsT=Xt8_4[:, k2, :, jt * P : (jt + 1) * P],
                    rhs=Xt8_4[:, k2, :, ccols],
                    perf_mode=DR,
                    start=(k2 == 0),
                    stop=(k2 == NK // 2 - 1),
                )
            nc.vector.tensor_add(ps[:], ps[:], negc_full[:, ccols])
            nc.scalar.activation(
                E_T[:, jt * S + ic * JC : jt * S + (ic + 1) * JC],
                ps[:],
                AF.Exp,
            )

        # ---- startup: process tiles as their batch DMAs land. negc for
        # chunk n is emitted right after tile 4n+3 so it's always ready
        # before any consumer; chunk 0's QK tiles interleave one tile
        # behind the loads so each QK tile's transposed operand has already
        # drained when the tensor engine reaches it. ----
        for i in range(NI):
            emit_load_tile(i)
            if i % NSUB == NSUB - 1:
                emit_sq_negc(i // NSUB)
            if i == NSUB:
                for jt in range(NSUB):
                    emit_qk_tile(0, jt)
            elif i > NSUB:
                emit_qk_tile(0, i - 1)
        emit_qk_tile(0, NI - 1)

        def emit_pv_steps(i, po, pl, j0, j1):
            for j in range(j0, j1):
                lhsT = E_T[:, j * S + i * P : j * S + (i + 1) * P]
                nc.tensor.matmul(
                    po[:],
                    lhsT=lhsT,
                    rhs=X_bf[:, j * D : (j + 1) * D],
                    start=(j == 0),
                    stop=(j == NI - 1),
                )
                nc.tensor.matmul(
                    pl[:],
                    lhsT=lhsT,
                    rhs=ones_col[:],
                    start=(j == 0),
                    stop=(j == NI - 1),
                )

        def emit_pv_end(i, po, pl):
            linv = stat_pool.tile([P, 1], f32, tag="linv", name=f"linv{i}")
            nc.vector.reciprocal(linv[:], pl[:])
            osb = osb_pool.tile([P, D], f32, tag="osb", name=f"osb{i}")
            nc.vector.tensor_scalar_mul(osb[:], po[:], linv[:])
            nc.sync.dma_start(out[i * P : (i + 1) * P, :], osb[:])

        # Main loop: chunk ic's QK tiles are interleaved with chunk ic-1's PV
        # matmuls at quarter-tile granularity, so the tensor engine always has
        # ready PV work queued while a QK psum bank waits on its add+exp
        # drain. The PV j-step order matches the exp completion order of the
        # previous chunk, so interleaved steps never wait on the softmax.
        # (Chunk 0's QK was emitted during the load stream above.)
        po = pl = None
        for ic in range(1, NC + 1):
            for jt in range(NI):
                if ic < NC:
                    emit_qk_tile(ic, jt)
                i = (ic - 1) * NSUB + jt // NSUB
                if jt % NSUB == 0:
                    po = pv_psum.tile([P, D], f32, tag="pv", name=f"pv{i}")
                    pl = l_psum.tile([P, 1], f32, tag="lc", name=f"l{i}")
                emit_pv_steps(i, po, pl, (jt % NSUB) * NSUB, (jt % NSUB + 1) * NSUB)
                if jt % NSUB == NSUB - 1:
                    emit_pv_end(i, po, pl)

    nc.compile()
    return nc


def _maybe_install_trace_hook():
    """Install the NTFF profile hook (test/profiling only; optional)."""
    import sys
    import types

    try:
        from antenv.axon_hooks import get_axon_ntff_profile_hook  # noqa: F401

        return  # already available
    except ImportError:
        pass
    try:
        mod = types.ModuleType("antenv.axon_hooks")
        _hook = [None]
        mod.set_axon_ntff_profile_hook = lambda h: _hook.__setitem__(0, h)
        mod.get_axon_ntff_profile_hook = lambda: _hook[0]
        sys.modules["antenv.axon_hooks"] = mod
        import antenv

        antenv.axon_hooks = mod
        from trn_agent_boot.trn_boot import _ntff_profile_via_ctypes

        mod.set_axon_ntff_profile_hook(
            _ntff_profile_via_ctypes("/opt/axon/libaxon_pjrt.so")
        )
    except Exception:
        pass


def kernel(inputs: np.ndarray) -> np.ndarray:
    from concourse.bass_utils import run_bass_kernel_spmd

    x = np.ascontiguousarray(np.asarray(inputs, dtype=np.float32))
    assert x.shape == (B, S, D), f"unexpected input shape {x.shape}"

    if "nc" not in _CACHE:
        _CACHE["nc"] = _build_nc()
    nc = _CACHE["nc"]

    trace = bool(int(os.environ.get("ATT_KERNEL_TRACE", "0")))
    if trace:
        _maybe_install_trace_hook()

    in_maps = [{"inputs": x[b]} for b in range(B)]
    res = run_bass_kernel_spmd(nc, in_maps, core_ids=list(range(B)), trace=trace)
    kernel.last_exec_time_ns = res.exec_time_ns
    return np.stack([res.results[b]["out"] for b in range(B)], axis=0)


kernel.last_exec_time_ns = None


# revision 10
# speedup vs baseline: 1.1422x; 1.0051x over previous
"""Self-attention layer (softmax(X @ X^T) @ X) on 8 Trainium2 NeuronCores.

Data-parallel over batch: each of the 8 cores computes one batch element's
full attention for X of shape [2048, 512].

Per-core algorithm. Scores are computed TRANSPOSED (T[j, i] = <x_i, x_j>,
key index j on partitions) so the exponentiated tile is already in the
layout the PV matmul needs as its stationary operand. QK matmuls run in
fp8-e4m3 DoubleRow (2 MACs/cell/cycle, K=256 per matmul) — score rounding
cancels exactly through the l-normalization. PV runs in bf16 for output
precision. The softmax stabilizer c[i] = |x_i|^2 is subtracted on the
vector engine from a pre-broadcast [128, S] row; l and 1/l come from an
N=1 matmul sharing the PV matmuls' loaded weights.

  1. Input streams in via 4 batched DMAs (4 row-tiles each) split across
     the Sync and Activation HWDGE queues, issued before any other work so
     the transfer pipeline fills during the framework preamble.
  2. Per tile (as its batch lands): X_bf = bf16(X) on gpsimd (keeps the
     scalar engine free for exp); PE-transpose X_bf into Xt8 (fp8, [d, s]).
  3. negc for chunk n is computed as soon as tiles 4n..4n+3 are in
     (sq = Xt8*Xt8; c row via ones-matmul over sq; broadcast -c into
     negc_full [128, S] via K=1 matmul + copy) so no QK tile ever waits on
     it mid-kernel.
  4. For each query chunk ic (512 queries), for each key block jt:
       psum = QK fp8 DoubleRow matmuls (2 x K=256)
       psum -= c (vector engine)
       E_T[jt block][:, ic] = exp(psum)   (scalar engine, direct to SBUF)
     qk_psum has 3 banks so the add+exp drain of a tile is fully hidden
     behind two tile periods of PE work.
  5. PV per query block i (software-pipelined one chunk behind QK):
       po = sum_j E_T[:, j-block, i]^T @ X_bf[j]   (bf16)
       l  = same loaded weights @ ones (N=1);  O_i = po / l; DMA out.
"""

import os
import numpy as np

B, S, D = 8, 2048, 512
P = 128
NI = S // P  # 16 row blocks
NK = D // P  # 4 d-tiles
JC = 512     # query column chunk (one psum bank)
NC = S // JC  # 4 chunks
NSUB = JC // P  # 4 i-tiles per chunk
LB = 2       # input tiles per batched DMA
NB = NI // LB  # 8 batched input DMAs

_CACHE = {}


def _build_nc():
    from contextlib import ExitStack

    import concourse.bacc as bacc
    import concourse.mybir as mybir
    import concourse.tile as tile
    from concourse import masks

    f32 = mybir.dt.float32
    bf16 = mybir.dt.bfloat16
    fp8 = mybir.dt.float8e4
    AF = mybir.ActivationFunctionType
    DR = mybir.MatmulPerfMode.DoubleRow

    nc = bacc.Bacc("TRN2", target_bir_lowering=False, debug=False, num_devices=B)
    inp = nc.dram_tensor("inputs", [S, D], f32, kind="ExternalInput").ap()
    out = nc.dram_tensor("out", [S, D], f32, kind="ExternalOutput").ap()

    with tile.TileContext(nc) as tc, ExitStack() as ctx:
        const_pool = ctx.enter_context(tc.tile_pool(name="const", bufs=1))
        persist = ctx.enter_context(tc.tile_pool(name="persist", bufs=1))
        stat_pool = ctx.enter_context(tc.tile_pool(name="stat", bufs=2))
        osb_pool = ctx.enter_context(tc.tile_pool(name="osb", bufs=2))
        # PSUM budget (8 banks): qk 3 + pt 1 + pv 3 + lc 1. Three QK banks
        # hide a tile's add+exp drain (~1.7us) behind two interleaved tile
        # periods (~2.8us); three PV banks keep po issue stall-free across
        # i-block boundaries. tr only carries the load-phase transposes and
        # negc scratch, which the DMA stream paces anyway.
        qk_psum = ctx.enter_context(tc.tile_pool(name="qk_psum", bufs=3, space="PSUM"))
        tr_psum = ctx.enter_context(tc.tile_pool(name="tr_psum", bufs=1, space="PSUM"))
        pv_psum = ctx.enter_context(tc.tile_pool(name="pv_psum", bufs=3, space="PSUM"))
        l_psum = ctx.enter_context(tc.tile_pool(name="l_psum", bufs=1, space="PSUM"))

        # X_f32 is the DMA landing buffer for the whole input; batched DMAs
        # are issued before anything else so transfers overlap the preamble.
        X_f32 = persist.tile([P, NI * D], f32, tag="xf32", name="xf32")
        Xf3 = X_f32[:].rearrange("p (t d) -> p t d", t=NI)
        inp3 = inp.rearrange("(t p) d -> p t d", t=NI)
        for b in range(NB):
            eng = nc.sync if b % 2 == 0 else nc.scalar
            eng.dma_start(
                Xf3[:, b * LB : (b + 1) * LB], inp3[:, b * LB : (b + 1) * LB]
            )

        # PE warm-up filler tile: HAM keeps the PE clock-gated at 1.2 GHz
        # until it sees ~3.4us of sustained matmul activity, and re-gates
        # after ~3.4us of idle; transpose-mode matmuls don't count. Dummy
        # N=128 matmuls on this memset tile are emitted LAST (lowest
        # scheduler priority) so they pad the PE's DMA-paced idle gaps
        # during the load phase, keeping the clock at 2.4 GHz throughout.
        warm = const_pool.tile([P, P], bf16, tag="warm", name="warm")
        nc.vector.memset(warm[:], 0.0)

        ident = const_pool.tile([P, P], bf16, tag="ident", name="ident")
        masks.make_identity(nc, ident[:])
        ones_row = const_pool.tile([1, P], bf16, tag="ones_row", name="ones_row")
        nc.vector.memset(ones_row[:], 1.0)
        ones_col = const_pool.tile([P, 1], bf16, tag="ones_col", name="ones_col")
        nc.vector.memset(ones_col[:], 1.0)

        X_bf = persist.tile([P, NI * D], bf16, tag="xbf", name="xbf")
        Xt8 = persist.tile([P, NK * S], fp8, tag="xt8", name="xt8")
        sq = persist.tile([P, NK * S], bf16, tag="sq", name="sq")
        negc = persist.tile([1, S], bf16, tag="negc", name="negc")
        negc_full = persist.tile([P, S], bf16, tag="negc_full", name="negc_full")
        E_T = persist.tile([P, NI * S], bf16, tag="et", name="et")

        Xt8_3 = Xt8[:].rearrange("p (k s) -> p k s", k=NK)
        Xt8_4 = Xt8[:].rearrange("p (k2 two s) -> p k2 two s", k2=NK // 2, two=2)
        sq3 = sq[:].rearrange("p (k s) -> p k s", k=NK)

        # ---- emit helpers ----
        def emit_load_tile(i):
            dcols = slice(i * D, (i + 1) * D)
            nc.scalar.copy(X_bf[:, dcols], Xf3[:, i])
            pt = tr_psum.tile([P, NK, P], bf16, tag="pt", name=f"ptx{i}")
            for k in range(NK):
                nc.tensor.matmul(
                    pt[:, k],
                    lhsT=X_bf[:, i * D + k * P : i * D + (k + 1) * P],
                    rhs=ident[:],
                    is_transpose=True,
                    skip_group_check=True,
                )
            nc.vector.tensor_copy(Xt8_3[:, :, i * P : (i + 1) * P], pt[:])

        def emit_sq_negc(ic):
            # c[s] = sum_d X[s, d]^2 for chunk ic's columns (psum row),
            # negated and broadcast to all partitions of negc_full
            ccols = slice(ic * JC, (ic + 1) * JC)
            for k in range(NK):
                eng = nc.gpsimd if k % 2 == 0 else nc.vector
                eng.tensor_mul(
                    sq3[:, k, ccols], Xt8_3[:, k, ccols], Xt8_3[:, k, ccols]
                )
            pc = tr_psum.tile([1, JC], f32, tag="pt", name=f"c{ic}")
            for k in range(NK):
                nc.tensor.matmul(
                    pc[:],
                    lhsT=ones_col[:],
                    rhs=sq3[:, k, ccols],
                    start=(k == 0),
                    stop=(k == NK - 1),
                )
            nc.vector.tensor_scalar_mul(negc[:, ccols], pc[:], -1.0)
            pb = tr_psum.tile([P, JC], f32, tag="pt", name=f"pb{ic}")
            nc.tensor.matmul(pb[:], lhsT=ones_row[:], rhs=negc[:, ccols])
            nc.vector.tensor_copy(negc_full[:, ccols], pb[:])

        def emit_qk_tile(ic, jt):
            ccols = slice(ic * JC, (ic + 1) * JC)
            ps = qk_psum.tile([P, JC], f32, tag="qk", name=f"qk{ic}_{jt}")
            for k2 in range(NK // 2):
                nc.tensor.matmul(
                    ps[:],
                    lhsT=Xt8_4[:, k2, :, jt * P : (jt + 1) * P],
                    rhs=Xt8_4[:, k2, :, ccols],
                    perf_mode=DR,
                    start=(k2 == 0),
                    stop=(k2 == NK // 2 - 1),
                )
            nc.vector.tensor_add(ps[:], ps[:], negc_full[:, ccols])
            nc.scalar.activation(
                E_T[:, jt * S + ic * JC : jt * S + (ic + 1) * JC],
                ps[:],
                AF.Exp,
            )

        # Warm-up psum lives in the l pool; it is allocated before any pl
        # tile so the first PV l-matmul simply waits for the (cheap) dummy
        # stream to finish rather than deadlocking on the slot.
        wps = l_psum.tile([P, P], f32, tag="lc", name="warmps")

        # ---- startup: process tiles as their batch DMAs land. negc for
        # chunk n is emitted right after tile 4n+3 so it's always ready
        # before any consumer; chunk 0's QK tiles interleave one tile
        # behind the loads so each QK tile's transposed operand has already
        # drained when the tensor engine reaches it. ----
        for i in range(NI):
            emit_load_tile(i)
            if i % NSUB == NSUB - 1:
                emit_sq_negc(i // NSUB)
            if i == NSUB:
                for jt in range(NSUB):
                    emit_qk_tile(0, jt)
            elif i > NSUB:
                emit_qk_tile(0, i - 1)
        emit_qk_tile(0, NI - 1)

        def emit_pv_steps(i, po, pl, j0, j1):
            for j in range(j0, j1):
                lhsT = E_T[:, j * S + i * P : j * S + (i + 1) * P]
                nc.tensor.matmul(
                    po[:],
                    lhsT=lhsT,
                    rhs=X_bf[:, j * D : (j + 1) * D],
                    start=(j == 0),
                    stop=(j == NI - 1),
                )
                nc.tensor.matmul(
                    pl[:],
                    lhsT=lhsT,
                    rhs=ones_col[:],
                    start=(j == 0),
                    stop=(j == NI - 1),
                )

        def emit_pv_end(i, po, pl):
            linv = stat_pool.tile([P, 1], f32, tag="linv", name=f"linv{i}")
            nc.vector.reciprocal(linv[:], pl[:])
            osb = osb_pool.tile([P, D], f32, tag="osb", name=f"osb{i}")
            nc.vector.tensor_scalar_mul(osb[:], po[:], linv[:])
            nc.sync.dma_start(out[i * P : (i + 1) * P, :], osb[:])

        # Main loop: chunk ic's QK tiles are interleaved with chunk ic-1's PV
        # matmuls at quarter-tile granularity, so the tensor engine always has
        # ready PV work queued while a QK psum bank waits on its add+exp
        # drain. The PV j-step order matches the exp completion order of the
        # previous chunk, so interleaved steps never wait on the softmax.
        # (Chunk 0's QK was emitted during the load stream above.)
        po = pl = None
        for ic in range(1, NC + 1):
            for jt in range(NI):
                if ic < NC:
                    emit_qk_tile(ic, jt)
                i = (ic - 1) * NSUB + jt // NSUB
                if jt % NSUB == 0:
                    po = pv_psum.tile([P, D], f32, tag="pv", name=f"pv{i}")
                    pl = l_psum.tile([P, 1], f32, tag="lc", name=f"l{i}")
                emit_pv_steps(i, po, pl, (jt % NSUB) * NSUB, (jt % NSUB + 1) * NSUB)
                if jt % NSUB == NSUB - 1:
                    emit_pv_end(i, po, pl)

        # Lowest-priority PE idle filler (see `warm` above).
        for w in range(64):
            nc.tensor.matmul(wps[:], lhsT=warm[:], rhs=warm[:], skip_group_check=True)

    nc.compile()
    return nc


def _maybe_install_trace_hook():
    """Install the NTFF profile hook (test/profiling only; optional)."""
    import sys
    import types

    try:
        from antenv.axon_hooks import get_axon_ntff_profile_hook  # noqa: F401

        return  # already available
    except ImportError:
        pass
    try:
        mod = types.ModuleType("antenv.axon_hooks")
        _hook = [None]
        mod.set_axon_ntff_profile_hook = lambda h: _hook.__setitem__(0, h)
        mod.get_axon_ntff_profile_hook = lambda: _hook[0]
        sys.modules["antenv.axon_hooks"] = mod
        import antenv

        antenv.axon_hooks = mod
        from trn_agent_boot.trn_boot import _ntff_profile_via_ctypes

        mod.set_axon_ntff_profile_hook(
            _ntff_profile_via_ctypes("/opt/axon/libaxon_pjrt.so")
        )
    except Exception:
        pass


def kernel(inputs: np.ndarray) -> np.ndarray:
    from concourse.bass_utils import run_bass_kernel_spmd

    x = np.ascontiguousarray(np.asarray(inputs, dtype=np.float32))
    assert x.shape == (B, S, D), f"unexpected input shape {x.shape}"

    if "nc" not in _CACHE:
        _CACHE["nc"] = _build_nc()
    nc = _CACHE["nc"]

    trace = bool(int(os.environ.get("ATT_KERNEL_TRACE", "0")))
    if trace:
        _maybe_install_trace_hook()

    in_maps = [{"inputs": x[b]} for b in range(B)]
    res = run_bass_kernel_spmd(nc, in_maps, core_ids=list(range(B)), trace=trace)
    kernel.last_exec_time_ns = res.exec_time_ns
    return np.stack([res.results[b]["out"] for b in range(B)], axis=0)


kernel.last_exec_time_ns = None
